# revision 1
# baseline (speedup 1.0000x reference)
"""Multi-head attention (B=2, N=2048, D=1024, H=16) on 8 NeuronCores.

Sharding: data-parallel over batch (cores 0-3 -> b=0, cores 4-7 -> b=1),
tensor-parallel over heads (4 heads per core; column-parallel QKV,
row-parallel proj). Each core emits a partial projection output
y_c = O_heads(c) @ proj_w[rows(c)]; the host sums the 4 partials per batch
and adds proj_b.

Per-core kernel (Bass/Tile, fp32 data, float32r matmuls):
  A) PE-transpose x -> xT; qT/kT (head-pair-major) and v (n-major,
     ones-augmented column for the softmax denominator).
  B) flash-style attention in transposed space:
       ST[m,n] = kT.T qT  (PSUM) -> exp(SCALE*st) on ACT -> SBUF
       U = [v|1].T E accumulated over m-tiles; row 64 of U is the
       softmax denominator; normalize with a fast DVE copy (early PSUM
       release) + reciprocal + DRAM-bounce broadcast + multiply into
       OT (c-major).
  C) y = OT.T @ wp_local (K=256 over the core's head channels).
"""

import numpy as np

import concourse.bass as bass
import concourse.tile as tile
from concourse import mybir
from concourse.bass_utils import run_bass_kernel_spmd
from concourse.masks import make_identity
from concourse import library_config

# ---- problem constants (hardcoded per contract) ----
B = 2
N = 2048
D = 1024
H = 16
HD = 64          # head dim
SCALE = HD ** -0.5
NC = 8           # cores
HL = H // (NC // B)   # heads per core = 4
CW = HL * HD     # local qkv column width = 256

F32 = mybir.dt.float32
F32R = mybir.dt.float32r

MM_DT = F32R     # matmul compute dtype (bitcast view)

NT = N // 128    # 16 n-tiles (also m-tiles)
KC = D // 128    # 8 contraction chunks for qkv matmuls


def _mm(ap):
    """View an fp32 AP as the matmul compute dtype."""
    if MM_DT is F32:
        return ap
    return ap.bitcast(MM_DT)


def _r(ap):
    """Output-cast: write rounded to the matmul compute dtype (the BIR
    verifier requires fp32r matmul operands to be produced rounded)."""
    if MM_DT is F32:
        return ap
    return ap.bitcast(MM_DT)


def _split_sync_waits(nc, maxw: int = 1) -> int:
    """This walrus build rejects >1 semaphore-wait per instruction
    (setupSyncWait: "Too many sync wait commands"). Hoist excess waits
    onto preceding same-engine no-ops: the sequencer runs instructions
    in order, so the semantics are unchanged."""
    n_split = 0
    for fn in nc.m.functions:
        for bb in fn.blocks:
            insts = list(bb.instructions)
            out = []
            changed = False
            for inst in insts:
                si = inst.sync_info
                waits = list(si.on_wait) if si is not None and si.on_wait else []
                if len(waits) > maxw:
                    chunks = [waits[i: i + maxw] for i in range(0, len(waits), maxw)]
                    for chunk in chunks[:-1]:
                        out.append(mybir.InstNoOp(
                            name=f"I-splitw-{nc.next_id()}",
                            sync_info=mybir.SyncInfo(on_wait=chunk, on_update=[]),
                            bass_nofuse=True,
                            engine=inst.engine,
                        ))
                    si.on_wait = chunks[-1]
                    inst.sync_info = si
                    n_split += 1
                    changed = True
                out.append(inst)
            if changed:
                try:
                    bb.instructions = out
                except Exception:
                    bb.instructions.clear()
                    for i in out:
                        bb.instructions.append(i)
    return n_split


def _build_program(split=True, reps=1, stages="ABC"):
    nc = bass.Bass(trn_type="TRN2", target_bir_lowering=False, debug=False)

    x_d = nc.dram_tensor("x", [N, D], F32, kind="ExternalInput").ap()
    wq_d = nc.dram_tensor("wq", [D, CW], F32, kind="ExternalInput").ap()
    wk_d = nc.dram_tensor("wk", [D, CW], F32, kind="ExternalInput").ap()
    wv_d = nc.dram_tensor("wv", [D, CW], F32, kind="ExternalInput").ap()
    wp_d = nc.dram_tensor("wp", [CW, D], F32, kind="ExternalInput").ap()
    qkvb_d = nc.dram_tensor("qkvb", [3 * CW], F32, kind="ExternalInput").ap()
    y_d = nc.dram_tensor("y", [N, D], F32, kind="ExternalOutput").ap()

    with tile.TileContext(nc) as tc:
        for rep in range(reps):
            rsc_d = nc.dram_tensor(f"rscratch{rep}", [16, 512], F32).ap()
            _body(nc, tc, x_d, wq_d, wk_d, wv_d, wp_d, qkvb_d, y_d, rsc_d,
                  stages=stages)

    if split:
        _split_sync_waits(nc)
    return nc


def _body(nc, tc, x_d, wq_d, wk_d, wv_d, wp_d, qkvb_d, y_d, rsc_d, stages="ABC"):
    from contextlib import ExitStack

    persist = ExitStack()
    const_p = persist.enter_context(tc.tile_pool(name="const", bufs=1))
    qk_p = persist.enter_context(tc.tile_pool(name="qk", bufs=1))
    v1_p = persist.enter_context(tc.tile_pool(name="v1", bufs=1))

    ident = const_p.tile([128, 128], F32)
    make_identity(nc, ident)

    qT = qk_p.tile([128, 2, N], F32)      # [row-in-pair, pair, n]
    kT = qk_p.tile([128, 2, N], F32)
    v1 = v1_p.tile([128, NT, HL, HD + 1], F32)   # ones in last column

    qb = const_p.tile([128, 2], F32)
    kb = const_p.tile([128, 2], F32)
    vbc = const_p.tile([128, CW], F32)
    for pair in range(2):
        nc.gpsimd.dma_start(qb[:, pair: pair + 1],
                            qkvb_d[bass.ds(pair * 128, 128)].unsqueeze(1))
        nc.gpsimd.dma_start(kb[:, pair: pair + 1],
                            qkvb_d[bass.ds(CW + pair * 128, 128)].unsqueeze(1))
    nc.gpsimd.dma_start(
        vbc,
        qkvb_d[bass.ds(2 * CW, CW)].unsqueeze(0).partition_broadcast(128).squeeze(1))

    # ones column of v1 (DVE memset cannot emit f32r; use in0*0 + 1)
    nc.vector.tensor_scalar(
        _r(v1[:, :, :, HD]),
        vbc[:, 0:NT * HL].rearrange("p (a b) -> p a b", a=NT),
        0.0, 1.0, mybir.AluOpType.mult, mybir.AluOpType.add)

    # ---------------- Stage A pools (right side: freed mid-kernel) --------
    sa = ExitStack()    # w + xT: alive until the last qk matmul
    sa1 = ExitStack()   # x staging + wv + wraw: freed earlier
    w_p = sa.enter_context(tc.tile_pool(name="w", bufs=1, side="right"))
    xT_p = sa.enter_context(tc.tile_pool(name="xT", bufs=1, side="right"))
    wv_p = sa1.enter_context(tc.tile_pool(name="wv", bufs=1, side="right"))
    wraw_p = sa1.enter_context(tc.tile_pool(name="wraw", bufs=1, side="right"))
    xs_p = sa1.enter_context(tc.tile_pool(name="xs", bufs=9, side="right"))

    # stage-A PSUM pool (right side, freed with sa1)
    ps_a = sa1.enter_context(tc.tile_pool(name="ps_a", bufs=2, space="PSUM",
                                          side="right"))

    wq_s = w_p.tile([128, KC, CW], F32)
    wk_s = w_p.tile([128, KC, CW], F32)
    wv_s = wv_p.tile([128, KC, CW], F32)

    def load_weights():
        for (wd, ws) in ((wv_d, wv_s), (wq_d, wq_s), (wk_d, wk_s)):
            wr = wraw_p.tile([128, KC, CW], F32, tag="wraw", name="wraw")
            nc.gpsimd.dma_start(wr, wd.rearrange("(t p) c -> p t c", p=128))
            nc.vector.tensor_copy(_r(ws), wr)

    xT = xT_p.tile([128, KC, N], F32)

    def tg_load(g):
        """g indexes groups of 4 n-tiles (512 rows)."""
        xts = []
        for i in range(4):
            xt = xs_p.tile([128, D], F32, tag="xs", name="xs")
            nc.sync.dma_start(xt, x_d[bass.ds((g * 4 + i) * 128, 128), :])
            xts.append(xt)
        return xts

    def tg_dc(xts, g, dc):
        pt = ps_a.tile([128, 512], F32, tag="pt", name="pt")
        for i in range(4):
            nc.tensor.transpose(
                pt[:, i * 128:(i + 1) * 128],
                xts[i][:, dc * 128:(dc + 1) * 128],
                ident)
        nc.scalar.activation(
            _r(xT[:, dc, bass.ds(g * 512, 512)]), pt,
            mybir.ActivationFunctionType.Identity)

    def emit_v(mt):
        ps = ps_a.tile([128, CW], F32, tag="psv", name="psv")
        for dc in range(KC):
            nc.tensor.matmul(
                ps,
                _mm(xT[:, dc, bass.ds(mt * 128, 128)]),
                _mm(wv_s[:, dc, :]),
                start=(dc == 0), stop=(dc == KC - 1))
        nc.vector.tensor_add(
            _r(v1[:, mt, :, 0:HD]),
            ps.rearrange("p (h d) -> p h d", h=HL),
            vbc.rearrange("p (h d) -> p h d", h=HL))

    def emit_qk(pair, which, nb4):
        wt, dst, bias = ((wq_s, qT, qb), (wk_s, kT, kb))[which]
        ps = ps_a.tile([128, 512], F32, tag="psqk", name="psqk")
        for dc in range(KC):
            nc.tensor.matmul(
                ps,
                _mm(wt[:, dc, bass.ds(pair * 128, 128)]),
                _mm(xT[:, dc, bass.ds(nb4 * 512, 512)]),
                start=(dc == 0), stop=(dc == KC - 1))
        nc.scalar.activation(
            _r(dst[:, pair, bass.ds(nb4 * 512, 512)]), ps,
            mybir.ActivationFunctionType.Identity,
            bias=bias[:, pair: pair + 1])

    # --- prefix: what block(p0, nb0) mts 0..7 needs -----------------------
    xts01 = [tg_load(0), tg_load(1)]
    load_weights()
    for g in (0, 1):
        for dc in range(KC):
            tg_dc(xts01[g], g, dc)
    for mt in range(8):
        emit_v(mt)
    for nb4 in (0, 1):
        emit_qk(0, 0, nb4)
        emit_qk(0, 1, nb4)
    xts2 = tg_load(2)
    xts3 = tg_load(3)

    def emit_a_rest():
        for g, xts in ((2, xts2), (3, xts3)):
            for dc in range(KC):
                tg_dc(xts, g, dc)
        for mt in range(8, NT):
            emit_v(mt)
        for nb4 in (2, 3):
            emit_qk(0, 0, nb4)
            emit_qk(0, 1, nb4)
        for nb4 in range(4):
            emit_qk(1, 0, nb4)
            emit_qk(1, 1, nb4)

    if "B" not in stages:
        emit_a_rest()
        sa1.close()
        sa.close()
        persist.close()
        return

    # ---------------- Stage B (attention) + C (proj) ----------------------
    emit_a_rest()
    sa1.close()
    sa.close()

    sb = ExitStack()
    et_p = sb.enter_context(tc.tile_pool(name="et", bufs=6))
    ps_st = sb.enter_context(tc.tile_pool(name="ps_st", bufs=2, space="PSUM"))
    ps_u = sb.enter_context(tc.tile_pool(name="ps_u", bufs=1, space="PSUM"))

    def make_us():
        us = {}
        for sub in range(2):
            for jc in range(2):
                us[(sub, jc)] = ps_u.tile([HD + 1, 512], F32,
                                          tag=f"u{sub}{jc}",
                                          name=f"u_{sub}_{jc}")
        return us

    def emit_block_part(pair, nb, us, mts):
        for mt in mts:
            for sub in range(2):
                st = ps_st.tile([128, 1024], F32, tag="st", name="st")
                for jc in range(2):
                    nc.tensor.matmul(
                        st[:, jc * 512:(jc + 1) * 512],
                        _mm(kT[bass.ds(sub * HD, HD), pair,
                               bass.ds(mt * 128, 128)]),
                        _mm(qT[bass.ds(sub * HD, HD), pair,
                               bass.ds(nb * 1024 + jc * 512, 512)]),
                        start=True, stop=True)
                et = et_p.tile([128, 1024], F32, tag="et", name="et")
                nc.scalar.activation(
                    _r(et), st, mybir.ActivationFunctionType.Exp,
                    scale=float(SCALE))
                for jc in range(2):
                    nc.tensor.matmul(
                        us[(sub, jc)],
                        _mm(v1[:, mt, pair * 2 + sub, :]),
                        _mm(et[:, jc * 512:(jc + 1) * 512]),
                        start=(mt == 0), stop=(mt == NT - 1))

    def emit_norm_reads(pair, nb, us, ri_p, rb_p, otu_p):
        work = []
        for sub in range(2):
            for jc in range(2):
                u = us[(sub, jc)]
                idx = ((nb * 2) + pair) * 4 + sub * 2 + jc
                # read u out quickly so the PSUM slot frees for the next block
                otu = otu_p.tile([HD, 512], F32, tag="otu", name="otu")
                nc.vector.tensor_copy(otu, u[0:HD, :])
                ri = ri_p.tile([1, 512], F32, tag="ri", name="ri")
                nc.vector.reciprocal(ri, u[HD:HD + 1, :])
                nc.sync.dma_start(rsc_d[idx: idx + 1, :], ri)
                rb = rb_p.tile([HD, 512], F32, tag="rb", name="rb")
                nc.sync.dma_start(
                    rb,
                    rsc_d[idx, :].unsqueeze(0)
                    .partition_broadcast(HD).squeeze(1))
                work.append((sub, jc, otu, rb))
        return work

    def emit_norm_muls(pair, nb, work, OT, jcs=(0, 1)):
        for (sub, jc, otu, rb) in work:
            if jc not in jcs:
                continue
            nc.vector.tensor_mul(
                _r(OT[bass.ds(sub * HD, HD), pair,
                      bass.ds(nb * 1024 + jc * 512, 512)]),
                otu, rb)

    def emit_normalize(pair, nb, us, ri_p, rb_p, otu_p, OT):
        work = emit_norm_reads(pair, nb, us, ri_p, rb_p, otu_p)
        emit_norm_muls(pair, nb, work, OT)

    us00 = make_us()
    emit_block_part(0, 0, us00, range(0, NT))

    # late pools (fit after xT/w are freed)
    ot_p = sb.enter_context(tc.tile_pool(name="ot", bufs=1))
    OT = ot_p.tile([128, 2, N], F32)   # [c-in-pair, pair, n]
    ri_p = sb.enter_context(tc.tile_pool(name="ri", bufs=4))
    rb_p = sb.enter_context(tc.tile_pool(name="rb", bufs=6))
    otu_p = sb.enter_context(tc.tile_pool(name="otu", bufs=6))
    y_p = sb.enter_context(tc.tile_pool(name="y", bufs=4))
    wp_p = sb.enter_context(tc.tile_pool(name="wp", bufs=1))
    wp_s = wp_p.tile([128, 2, D], F32)
    wp_raw = wp_p.tile([128, 2, D], F32)
    nc.gpsimd.dma_start(wp_raw, wp_d.rearrange("(t p) e -> p t e", p=128))
    nc.vector.tensor_copy(_r(wp_s), wp_raw)

    emit_normalize(0, 0, us00, ri_p, rb_p, otu_p, OT)

    def emit_proj(nt):
        yt = y_p.tile([128, D], F32, tag="y", name="y")
        for ec in range(2):
            ps = ps_u.tile([128, 512], F32,
                           tag=f"u{nt % 2}{ec}",
                           name=f"psy_{nt}_{ec}")
            for pair in range(2):
                nc.tensor.matmul(
                    ps,
                    _mm(OT[:, pair, bass.ds(nt * 128, 128)]),
                    _mm(wp_s[:, pair, bass.ds(ec * 512, 512)]),
                    start=(pair == 0), stop=(pair == 1))
            nc.vector.tensor_copy(yt[:, bass.ds(ec * 512, 512)], ps)
        nc.sync.dma_start(y_d[bass.ds(nt * 128, 128), :], yt)

    for (pair, nb) in ((1, 0), (0, 1)):
        us = make_us()
        emit_block_part(pair, nb, us, range(NT))
        emit_normalize(pair, nb, us, ri_p, rb_p, otu_p, OT)
        if "C" in stages and (pair, nb) == (0, 1):
            for nt in range(0, NT // 2):
                emit_proj(nt)
    us = make_us()
    emit_block_part(1, 1, us, range(NT))
    work = emit_norm_reads(1, 1, us, ri_p, rb_p, otu_p)
    emit_norm_muls(1, 1, work, OT, jcs=(0,))
    if "C" in stages:
        for nt in range(NT // 2, NT // 2 + 4):
            emit_proj(nt)
    emit_norm_muls(1, 1, work, OT, jcs=(1,))
    if "C" in stages:
        for nt in range(NT // 2 + 4, NT):
            emit_proj(nt)

    sb.close()
    persist.close()


_NC_CACHE = None


def _get_program():
    global _NC_CACHE
    if _NC_CACHE is None:
        _NC_CACHE = _build_program()
    return _NC_CACHE


def make_in_maps(x, qkv_w, qkv_b, proj_w):
    in_maps = []
    for c in range(NC):
        b, j = divmod(c, NC // B)
        cs = j * CW
        in_maps.append({
            "x": np.ascontiguousarray(x[b], np.float32),
            "wq": np.ascontiguousarray(qkv_w[:, cs: cs + CW], np.float32),
            "wk": np.ascontiguousarray(qkv_w[:, D + cs: D + cs + CW], np.float32),
            "wv": np.ascontiguousarray(qkv_w[:, 2 * D + cs: 2 * D + cs + CW], np.float32),
            "wp": np.ascontiguousarray(proj_w[cs: cs + CW, :], np.float32),
            "qkvb": np.concatenate([
                qkv_b[cs: cs + CW],
                qkv_b[D + cs: D + cs + CW],
                qkv_b[2 * D + cs: 2 * D + cs + CW]]).astype(np.float32),
        })
    return in_maps


def combine_outputs(results, proj_b):
    out = np.empty((B, N, D), np.float32)
    per = NC // B
    for b in range(B):
        acc = results[b * per]["y"].astype(np.float32)
        for c in range(b * per + 1, (b + 1) * per):
            acc = acc + results[c]["y"]
        out[b] = acc + proj_b[None, :].astype(np.float32)
    return out


def kernel(**inputs):
    x = np.asarray(inputs["x"], np.float32)
    qkv_w = np.asarray(inputs["qkv_w"], np.float32)
    qkv_b = np.asarray(inputs["qkv_b"], np.float32)
    proj_w = np.asarray(inputs["proj_w"], np.float32)
    proj_b = np.asarray(inputs["proj_b"], np.float32)

    nc = _get_program()
    in_maps = make_in_maps(x, qkv_w, qkv_b, proj_w)
    res = run_bass_kernel_spmd(nc, in_maps, list(range(NC)), trace=False)
    return combine_outputs(res.results, proj_b)



# revision 30
# speedup vs baseline: 1.6527x; 1.6527x over previous
"""Multi-head attention (B=2, N=2048, D=1024, H=16) on 8 NeuronCores.

Sharding: data-parallel over batch (cores 0-3 -> b=0, cores 4-7 -> b=1),
tensor-parallel over heads (4 heads per core; column-parallel QKV,
row-parallel proj). Each core emits a partial projection output
y_c = O_heads(c) @ proj_w[rows(c)]; the host sums the 4 partials per batch
and adds proj_b.

v2 design (bf16 matmuls; PE-floor ~164us, ACT exp ~135us):
  - x is cast to bf16 by a gpsimd (SWDGE) DMA and transposed by the DMA
    XBAR (InstDmaTransposeAnt) straight into the d-major xT layout; the
    PE does no transposes at all.
  - All matmuls run in bf16 (1 cycle/row, same as fp32r, half the SBUF).
  - Softmax normalization: the denominator row (65th row of the U
    accumulator) is reciprocal'd on DVE into SBUF, broadcast to 64 rows
    by a tiny ones-stationary PE matmul into the unused partitions
    64..127 of the same PSUM bank, then one DVE multiply produces the
    normalized OT tile. No DRAM bounce, no extra PSUM banks.
  - Emission interleaves leftover stage-A matmuls and projection tiles
    into the attention mt-loops as PE filler so the PE never idles while
    the ACT engine paces the exp stream.
  - Rep-level software pipelining: rep r+1's prologue (DMAs, casts,
    XBAR transposes) is EMITTED at the start of rep r's last block, so
    its queue positions precede rep r's tail and the PE stream crosses
    the rep boundary without a bubble. Tiles read until a rep's end
    (qT/kT, v1, OT, consts) are double-buffered.
"""

import numpy as np

import concourse.bass as bass
import concourse.tile as tile
from concourse import mybir
from concourse.bass_utils import run_bass_kernel_spmd

# ---- problem constants (hardcoded per contract) ----
B = 2
N = 2048
D = 1024
H = 16
HD = 64          # head dim
SCALE = HD ** -0.5
NC = 8           # cores
HL = H // (NC // B)   # heads per core = 4
CW = HL * HD     # local qkv column width = 256

F32 = mybir.dt.float32
F32R = mybir.dt.float32r
BF = mybir.dt.bfloat16

NT = N // 128    # 16 n-tiles (also m-tiles)
KC = D // 128    # 8 contraction chunks for qkv matmuls

ALL4 = [(s, j) for s in range(2) for j in range(2)]


def _split_sync_waits(nc, maxw: int = 1) -> int:
    """This walrus build rejects >1 semaphore-wait per instruction
    (setupSyncWait: "Too many sync wait commands"). Hoist excess waits
    onto preceding same-engine no-ops: the sequencer runs instructions
    in order, so the semantics are unchanged."""
    n_split = 0
    for fn in nc.m.functions:
        for bb in fn.blocks:
            insts = list(bb.instructions)
            out = []
            changed = False
            for inst in insts:
                si = inst.sync_info
                waits = list(si.on_wait) if si is not None and si.on_wait else []
                if len(waits) > maxw:
                    chunks = [waits[i: i + maxw] for i in range(0, len(waits), maxw)]
                    for chunk in chunks[:-1]:
                        out.append(mybir.InstNoOp(
                            name=f"I-splitw-{nc.next_id()}",
                            sync_info=mybir.SyncInfo(on_wait=chunk, on_update=[]),
                            bass_nofuse=True,
                            engine=inst.engine,
                        ))
                    si.on_wait = chunks[-1]
                    inst.sync_info = si
                    n_split += 1
                    changed = True
                out.append(inst)
            if changed:
                try:
                    bb.instructions = out
                except Exception:
                    bb.instructions.clear()
                    for i in out:
                        bb.instructions.append(i)
    return n_split


def _build_program(split=True, reps=1, stages="ABC"):
    from contextlib import ExitStack

    nc = bass.Bass(trn_type="TRN2", target_bir_lowering=False, debug=False)

    dram = dict(
        x=nc.dram_tensor("x", [N, D], F32, kind="ExternalInput").ap(),
        wq=nc.dram_tensor("wq", [D, CW], F32, kind="ExternalInput").ap(),
        wk=nc.dram_tensor("wk", [D, CW], F32, kind="ExternalInput").ap(),
        wv=nc.dram_tensor("wv", [D, CW], F32, kind="ExternalInput").ap(),
        wp=nc.dram_tensor("wp", [CW, D], F32, kind="ExternalInput").ap(),
        qkvb=nc.dram_tensor("qkvb", [3 * CW], F32, kind="ExternalInput").ap(),
        y=nc.dram_tensor("y", [N, D], F32, kind="ExternalOutput").ap(),
    )

    with tile.TileContext(nc) as tc:
        es = ExitStack()
        P = dict(
            const=es.enter_context(tc.tile_pool(name="const", bufs=2)),
            qk=es.enter_context(tc.tile_pool(name="qk", bufs=2)),
            v1=es.enter_context(tc.tile_pool(name="v1", bufs=2)),
            ot=es.enter_context(tc.tile_pool(name="ot", bufs=2)),
            w=es.enter_context(tc.tile_pool(name="w", bufs=1)),
            xT=es.enter_context(tc.tile_pool(name="xT", bufs=2)),
            xg=es.enter_context(tc.tile_pool(name="xg", bufs=2)),
            et=es.enter_context(tc.tile_pool(name="et", bufs=6)),
            ri=es.enter_context(tc.tile_pool(name="ri", bufs=4)),
            otu=es.enter_context(tc.tile_pool(name="otu", bufs=6)),
            y=es.enter_context(tc.tile_pool(name="y", bufs=3)),
            st=es.enter_context(tc.tile_pool(name="ps_st", bufs=2, space="PSUM")),
            us=es.enter_context(tc.tile_pool(name="ps_u", bufs=1, space="PSUM")),
        )

        S = _prologue(nc, P, dram)
        carry = None
        for r in range(reps):
            holder = {}

            def cb(holder=holder, last=(r == reps - 1)):
                if not last:
                    holder["S"] = _prologue(nc, P, dram)

            carry = _compute(nc, P, dram, S, cb, carry, stages)
            S = holder.get("S")
        if carry is not None:
            _final_tail(carry, stages)
        es.close()

    if split:
        _split_sync_waits(nc)
    return nc


def _prologue(nc, P, dram):
    """Allocate one rep's tiles and emit its DMAs/memsets (no PE work)."""
    S = {}
    S["qT"] = P["qk"].tile([128, 2, N], BF, tag="qT", name="qT")
    S["kT"] = P["qk"].tile([128, 2, N], BF, tag="kT", name="kT")
    S["v1"] = P["v1"].tile([128, NT, HL, HD + 1], BF, tag="v1", name="v1")
    S["OT"] = P["ot"].tile([128, 2, N], BF, tag="OT", name="OT")
    S["wq"] = P["w"].tile([128, KC, CW], BF, tag="wq", name="wq_s")
    S["wk"] = P["w"].tile([128, KC, CW], BF, tag="wk", name="wk_s")
    S["wv"] = P["w"].tile([128, KC, CW], BF, tag="wv", name="wv_s")
    S["wp"] = P["w"].tile([128, 2, D], BF, tag="wp", name="wp_s")
    S["qb"] = P["const"].tile([128, 2], F32, tag="qb", name="qb")
    S["kb"] = P["const"].tile([128, 2], F32, tag="kb", name="kb")
    S["vbc"] = P["const"].tile([128, CW], F32, tag="vbc", name="vbc")
    S["ones64"] = P["const"].tile([1, HD], BF, tag="ones", name="ones64")
    warm_in = P["const"].tile([1, 8], F32, tag="wi", name="warm_in")
    warm_out = P["const"].tile([1, 8], F32, tag="wo", name="warm_out")
    S["xT"] = P["xT"].tile([128, KC, N], BF, tag="xT", name="xT")
    xgs = [P["xg"].tile([128, 4, D], BF, tag="xg", name=f"xg{g}")
           for g in range(4)]

    # memsets + act-table warm (off the critical path)
    nc.gpsimd.memset(warm_in, 0.0)
    nc.scalar.activation(warm_out, warm_in,
                         mybir.ActivationFunctionType.Exp)
    nc.gpsimd.memset(S["ones64"], 1.0)
    nc.gpsimd.memset(S["v1"][:, :, :, HD], 1.0)

    # x group casts (f32 -> bf16) + weight casts share the gpsimd SWDGE
    # queue; order so the first-needed tensors land first.
    x_d = dram["x"]

    def emit_cast(g):
        nc.gpsimd.dma_start(
            xgs[g],
            x_d[bass.ds(g * 512, 512), :].rearrange("(t p) c -> p t c", p=128))

    def emit_xbar(g):
        for i in range(4):
            nt = g * 4 + i
            nc.sync.dma_start_transpose(
                S["xT"][:, :, bass.ds(nt * 128, 128)], xgs[g][:, i, :])

    emit_cast(0)
    for (key, ws) in (("wq", S["wq"]), ("wk", S["wk"]), ("wv", S["wv"])):
        nc.gpsimd.dma_start(ws, dram[key].rearrange("(t p) c -> p t c", p=128))
    emit_cast(1)
    emit_cast(2)
    emit_cast(3)
    # wp is read until the END of this rep's tail (which rides the next
    # rep's B00), so its cast can block late; keep it last on the queue.
    nc.gpsimd.dma_start(S["wp"], dram["wp"].rearrange("(t p) e -> p t e", p=128))

    emit_xbar(0)
    qkvb_d = dram["qkvb"]
    for pair in range(2):
        nc.sync.dma_start(S["qb"][:, pair: pair + 1],
                          qkvb_d[bass.ds(pair * 128, 128)].unsqueeze(1))
        nc.sync.dma_start(S["kb"][:, pair: pair + 1],
                          qkvb_d[bass.ds(CW + pair * 128, 128)].unsqueeze(1))
    nc.sync.dma_start(
        S["vbc"],
        qkvb_d[bass.ds(2 * CW, CW)].unsqueeze(0).partition_broadcast(128).squeeze(1))
    emit_xbar(1)
    emit_xbar(2)
    emit_xbar(3)
    return S


def _final_tail(carry, stages="ABC"):
    """Standalone tail for the last rep: normalize its B11 (jc-split) and
    emit the remaining projection tiles."""
    carry["bc"]([(0, 0), (1, 0)])
    carry["muls"]([(0, 0), (1, 0)])
    carry["bc"]([(0, 1), (1, 1)])
    if "C" in stages:
        for nt in range(NT // 2, NT // 2 + 4):
            carry["proj"](nt)
    carry["muls"]([(0, 1), (1, 1)])
    if "C" in stages:
        for nt in range(NT // 2 + 4, NT):
            carry["proj"](nt)


def _compute(nc, P, dram, S, next_prologue_cb, carry_in, stages="ABC"):
    qT, kT, v1, OT = S["qT"], S["kT"], S["v1"], S["OT"]
    wq_s, wk_s, wv_s, wp_s = S["wq"], S["wk"], S["wv"], S["wp"]
    qb, kb, vbc, ones64, xT = S["qb"], S["kb"], S["vbc"], S["ones64"], S["xT"]
    y_d = dram["y"]

    # ---------------- stage-A emitters ----------------
    def emit_v(mt):
        ps = P["st"].tile([128, 1024], F32, tag="st", name="psv")
        for dc in range(KC):
            nc.tensor.matmul(
                ps[:, 0:CW],
                xT[:, dc, bass.ds(mt * 128, 128)],
                wv_s[:, dc, :],
                start=(dc == 0), stop=(dc == KC - 1))
        nc.vector.tensor_add(
            v1[:, mt, :, 0:HD],
            ps[:, 0:CW].rearrange("p (h d) -> p h d", h=HL),
            vbc.rearrange("p (h d) -> p h d", h=HL))

    def emit_qk(pair, which, nb4):
        wt, dst, bias = ((wq_s, qT, qb), (wk_s, kT, kb))[which]
        ps = P["st"].tile([128, 1024], F32, tag="st", name="psqk")
        for dc in range(KC):
            nc.tensor.matmul(
                ps[:, 0:512],
                wt[:, dc, bass.ds(pair * 128, 128)],
                xT[:, dc, bass.ds(nb4 * 512, 512)],
                start=(dc == 0), stop=(dc == KC - 1))
        nc.vector.tensor_scalar(
            dst[:, pair, bass.ds(nb4 * 512, 512)], ps[:, 0:512],
            bias[:, pair: pair + 1], None, mybir.AluOpType.add)

    # ---------------- stage-B/C emitters ----------------
    def make_us():
        us = {}
        for sub in range(2):
            for jc in range(2):
                us[(sub, jc)] = P["us"].tile([128, 512], F32,
                                             tag=f"u{sub}{jc}",
                                             name=f"u_{sub}_{jc}")
        return us

    def emit_st_exp(pair, nb, mt):
        ets = []
        for sub in range(2):
            st = P["st"].tile([128, 1024], F32, tag="st", name="st")
            for jc in range(2):
                nc.tensor.matmul(
                    st[:, jc * 512:(jc + 1) * 512],
                    kT[bass.ds(sub * HD, HD), pair, bass.ds(mt * 128, 128)],
                    qT[bass.ds(sub * HD, HD), pair,
                       bass.ds(nb * 1024 + jc * 512, 512)],
                    start=True, stop=True)
            et = P["et"].tile([128, 1024], BF, tag="et", name="et")
            nc.scalar.activation(
                et, st, mybir.ActivationFunctionType.Exp, scale=float(SCALE))
            ets.append(et)
        return ets

    def emit_u(pair, us, mt, ets):
        for sub in range(2):
            for jc in range(2):
                nc.tensor.matmul(
                    us[(sub, jc)][0:HD + 1, :],
                    v1[:, mt, pair * 2 + sub, :],
                    ets[sub][:, jc * 512:(jc + 1) * 512],
                    start=(mt == 0), stop=(mt == NT - 1))

    def emit_recips(us, ris, otus):
        """Reciprocal of the denominator rows + copy of the numerator
        rows to SBUF (the DVE may read only ONE operand from PSUM, so
        the normalize multiply needs the numerators in SBUF)."""
        for sub in range(2):
            for jc in range(2):
                ri = P["ri"].tile([1, 512], BF, tag="ri", name="ri")
                with nc.allow_low_precision(reason="bf16 reciprocal feeds the bf16 broadcast matmul"):
                    nc.vector.reciprocal(ri, us[(sub, jc)][HD:HD + 1, :])
                ris[(sub, jc)] = ri
                otu = P["otu"].tile([HD, 512], F32, tag="otu", name="otu")
                nc.vector.tensor_copy(otu, us[(sub, jc)][0:HD, :])
                otus[(sub, jc)] = otu

    def emit_bc(us, ris, keys):
        for key in keys:
            nc.tensor.matmul(
                us[key][HD:128, :], ones64, ris[key],
                start=True, stop=True)

    def emit_norm_muls(pair, nb, us, otus, keys):
        for (sub, jc) in keys:
            nc.vector.tensor_mul(
                OT[bass.ds(sub * HD, HD), pair,
                   bass.ds(nb * 1024 + jc * 512, 512)],
                otus[(sub, jc)], us[(sub, jc)][HD:128, :])

    def emit_proj(nt):
        ps = P["st"].tile([128, 1024], F32, tag="st", name="psy")
        yt = P["y"].tile([128, D], F32, tag="y", name="y")
        for ec in range(2):
            for pair in range(2):
                nc.tensor.matmul(
                    ps[:, bass.ds(ec * 512, 512)],
                    OT[:, pair, bass.ds(nt * 128, 128)],
                    wp_s[:, pair, bass.ds(ec * 512, 512)],
                    start=(pair == 0), stop=(pair == 1))
            nc.vector.tensor_copy(yt[:, bass.ds(ec * 512, 512)],
                                  ps[:, bass.ds(ec * 512, 512)])
        nc.sync.dma_start(y_d[bass.ds(nt * 128, 128), :], yt)

    # ---------------- emission schedule ----------------
    # PE prefix: minimum needed for block (p0, nb0) to start; ordered to
    # match DMA arrival (g0+wq first, then wk, wv, g1).
    emit_qk(0, 0, 0)   # qT p0 n<512   (g0 + wq)
    emit_qk(0, 1, 0)   # kT p0 m<512   (g0 + wk)
    emit_v(0)          # v1 mt0        (g0 + wv)
    emit_qk(0, 0, 1)   # qT p0 n<1024  (g1)

    if "B" not in stages:
        for mt in range(1, NT):
            emit_v(mt)
        for nb4 in range(2, 4):
            emit_qk(0, 0, nb4)
        for nb4 in range(1, 4):
            emit_qk(0, 1, nb4)
        for which in range(2):
            for nb4 in range(4):
                emit_qk(1, which, nb4)
        next_prologue_cb()
        return None

    def block(pair, nb, us, tagged, filler, prev=None):
        """prev = carry dict of the previous block ('bc'/'muls'
        closures). Its normalization is pipelined into this block's
        first two mts:
        ST(mt0) -> bc(prev) -> ST(mt1) -> muls(prev) -> U(mt0) -> U(mt1).
        bc must precede U(mt0) on the PE queue (U waits on the muls,
        which wait on bc) and the extra ST gives the DVE time to finish
        the muls before U(mt0) needs the banks."""
        fi = 0

        def drain_tags(mt, cls):
            for ent in list(tagged):
                if ent[0] <= mt and ent[2] == cls:
                    tagged.remove(ent)
                    ent[1]()

        # U(k) is emitted after ST(k+2) so the PE never sits behind the
        # ACT exp latency in its own FIFO (st bufs=2 and et bufs cover
        # the 2-mt lag). Each iteration emits the ST as early as
        # possible (ACT is the pacing engine); only 'st'-class tags
        # (kT chunks, needed by the ST itself) go before it.
        pend = {}
        drain_tags(0, "st")
        pend[0] = emit_st_exp(pair, nb, 0)
        if prev is not None:
            prev["bc"](ALL4)
        drain_tags(0, "u")
        drain_tags(1, "st")
        pend[1] = emit_st_exp(pair, nb, 1)
        if prev is not None:
            prev["muls"](ALL4)
        drain_tags(1, "u")
        for mt in range(2, NT):
            drain_tags(mt, "st")
            pend[mt] = emit_st_exp(pair, nb, mt)
            emit_u(pair, us, mt - 2, pend.pop(mt - 2))
            drain_tags(mt, "u")
            # pure filler: pace the remaining atoms over remaining mts
            if filler:
                remaining_mts = NT - mt
                pace = max(1, (len(filler) - fi + remaining_mts - 1)
                           // remaining_mts)
                for _ in range(pace):
                    if fi < len(filler):
                        filler[fi]()
                        fi += 1
        while fi < len(filler):
            filler[fi]()
            fi += 1
        emit_u(pair, us, NT - 2, pend.pop(NT - 2))
        emit_u(pair, us, NT - 1, pend.pop(NT - 1))
        ris = {}
        otus = {}
        emit_recips(us, ris, otus)
        return ris, otus

    def mk_carry(pair, nb, us, ris, otus):
        return {
            "bc": lambda keys: emit_bc(us, ris, keys),
            "muls": lambda keys: emit_norm_muls(pair, nb, us, otus, keys),
            "proj": emit_proj,
        }

    # ---- B00: (pair0, nb0) ----
    # carry_in is the PREVIOUS rep's B11: its normalization rides this
    # block's first two mts and its tail projections (n-tiles 8..15 of
    # the previous rep) are this block's filler.
    us00 = make_us()
    tagged00 = []
    for j in range(1, NT):
        tagged00.append((j, (lambda m=j: emit_v(m)), "u"))
    for k in range(1, 4):
        tagged00.append((4 * k, (lambda n=k: emit_qk(0, 1, n)), "st"))
    tagged00.sort(key=lambda t: t[0])
    filler00 = []
    if carry_in is not None and "C" in stages:
        filler00 += [(lambda n=nt, p=carry_in["proj"]: p(n))
                     for nt in range(NT // 2, NT)]
    filler00 += [lambda: emit_qk(0, 0, 2), lambda: emit_qk(0, 0, 3)]
    ris00, otus00 = block(0, 0, us00, tagged00, filler00, prev=carry_in)

    # ---- B01: (pair0, nb1) ----
    us01 = make_us()
    filler01 = [lambda: emit_qk(1, 0, 0), lambda: emit_qk(1, 0, 1),
                lambda: emit_qk(1, 1, 0), lambda: emit_qk(1, 1, 1)]
    ris01, otus01 = block(0, 1, us01, [], filler01,
                          prev=mk_carry(0, 0, us00, ris00, otus00))

    # ---- B10: (pair1, nb0) ----
    us10 = make_us()
    tagged10 = [(8, lambda: emit_qk(1, 1, 2), "st"),
                (12, lambda: emit_qk(1, 1, 3), "st")]
    filler10 = [lambda: emit_qk(1, 0, 2), lambda: emit_qk(1, 0, 3)]
    ris10, otus10 = block(1, 0, us10, tagged10, filler10,
                          prev=mk_carry(0, 1, us01, ris01, otus01))

    # ---- B11: (pair1, nb1) ----
    # Emit the NEXT rep's prologue first so its DMA-queue positions
    # precede this rep's tail and the rep boundary pipelines.
    next_prologue_cb()
    us11 = make_us()
    filler11 = [(lambda n=nt: emit_proj(n)) for nt in range(NT // 2)] \
        if "C" in stages else []
    ris11, otus11 = block(1, 1, us11, [], filler11,
                          prev=mk_carry(1, 0, us10, ris10, otus10))

    # prefix matmuls of the NEXT rep would normally go here in the PE
    # stream; they are emitted by the next _compute call right after we
    # return, which is the same queue position.
    return mk_carry(1, 1, us11, ris11, otus11)


_NC_CACHE = None


def _get_program():
    global _NC_CACHE
    if _NC_CACHE is None:
        _NC_CACHE = _build_program()
    return _NC_CACHE


def make_in_maps(x, qkv_w, qkv_b, proj_w):
    in_maps = []
    for c in range(NC):
        b, j = divmod(c, NC // B)
        cs = j * CW
        in_maps.append({
            "x": np.ascontiguousarray(x[b], np.float32),
            "wq": np.ascontiguousarray(qkv_w[:, cs: cs + CW], np.float32),
            "wk": np.ascontiguousarray(qkv_w[:, D + cs: D + cs + CW], np.float32),
            "wv": np.ascontiguousarray(qkv_w[:, 2 * D + cs: 2 * D + cs + CW], np.float32),
            "wp": np.ascontiguousarray(proj_w[cs: cs + CW, :], np.float32),
            "qkvb": np.concatenate([
                qkv_b[cs: cs + CW],
                qkv_b[D + cs: D + cs + CW],
                qkv_b[2 * D + cs: 2 * D + cs + CW]]).astype(np.float32),
        })
    return in_maps


def combine_outputs(results, proj_b):
    out = np.empty((B, N, D), np.float32)
    per = NC // B
    for b in range(B):
        acc = results[b * per]["y"].astype(np.float32)
        for c in range(b * per + 1, (b + 1) * per):
            acc = acc + results[c]["y"]
        out[b] = acc + proj_b[None, :].astype(np.float32)
    return out


def kernel(**inputs):
    x = np.asarray(inputs["x"], np.float32)
    qkv_w = np.asarray(inputs["qkv_w"], np.float32)
    qkv_b = np.asarray(inputs["qkv_b"], np.float32)
    proj_w = np.asarray(inputs["proj_w"], np.float32)
    proj_b = np.asarray(inputs["proj_b"], np.float32)

    nc = _get_program()
    in_maps = make_in_maps(x, qkv_w, qkv_b, proj_w)
    res = run_bass_kernel_spmd(nc, in_maps, list(range(NC)), trace=False)
    return combine_outputs(res.results, proj_b)


# revision 31
# speedup vs baseline: 3.7074x; 2.2432x over previous
"""Multi-head attention (B=2, N=2048, D=1024, H=16) on 8 NeuronCores.

Sharding: data-parallel over batch (cores 0-3 -> b=0, cores 4-7 -> b=1),
tensor-parallel over heads (4 heads per core; column-parallel QKV,
row-parallel proj). Each core emits a partial projection output
y_c = O_heads(c) @ proj_w[rows(c)]; the host sums the 4 partials per batch
and adds proj_b.

v2 design (bf16 matmuls; PE-floor ~164us, ACT exp ~135us):
  - x is cast to bf16 by a gpsimd (SWDGE) DMA and transposed by the DMA
    XBAR (InstDmaTransposeAnt) straight into the d-major xT layout; the
    PE does no transposes at all.
  - All matmuls run in bf16 (1 cycle/row, same as fp32r, half the SBUF).
  - Softmax normalization: the denominator row (65th row of the U
    accumulator) is reciprocal'd on DVE into SBUF, broadcast to 64 rows
    by a tiny ones-stationary PE matmul into the unused partitions
    64..127 of the same PSUM bank, then one DVE multiply produces the
    normalized OT tile. No DRAM bounce, no extra PSUM banks.
  - Emission interleaves leftover stage-A matmuls and projection tiles
    into the attention mt-loops as PE filler so the PE never idles while
    the ACT engine paces the exp stream.
  - Rep-level software pipelining: rep r+1's prologue (DMAs, casts,
    XBAR transposes) is EMITTED at the start of rep r's last block, so
    its queue positions precede rep r's tail and the PE stream crosses
    the rep boundary without a bubble. Tiles read until a rep's end
    (qT/kT, v1, OT, consts) are double-buffered.
"""

import numpy as np

import concourse.bass as bass
import concourse.tile as tile
from concourse import mybir
from concourse.bass_utils import run_bass_kernel_spmd

# ---- problem constants (hardcoded per contract) ----
B = 2
N = 2048
D = 1024
H = 16
HD = 64          # head dim
SCALE = HD ** -0.5
NC = 8           # cores
HL = H // (NC // B)   # heads per core = 4
CW = HL * HD     # local qkv column width = 256

F32 = mybir.dt.float32
F32R = mybir.dt.float32r
BF = mybir.dt.bfloat16

NT = N // 128    # 16 n-tiles (also m-tiles)
KC = D // 128    # 8 contraction chunks for qkv matmuls

ALL4 = [(s, j) for s in range(2) for j in range(2)]


def _split_sync_waits(nc, maxw: int = 1) -> int:
    """This walrus build rejects >1 semaphore-wait per instruction
    (setupSyncWait: "Too many sync wait commands"). Hoist excess waits
    onto preceding same-engine no-ops: the sequencer runs instructions
    in order, so the semantics are unchanged."""
    n_split = 0
    for fn in nc.m.functions:
        for bb in fn.blocks:
            insts = list(bb.instructions)
            out = []
            changed = False
            for inst in insts:
                si = inst.sync_info
                waits = list(si.on_wait) if si is not None and si.on_wait else []
                if len(waits) > maxw:
                    chunks = [waits[i: i + maxw] for i in range(0, len(waits), maxw)]
                    for chunk in chunks[:-1]:
                        out.append(mybir.InstNoOp(
                            name=f"I-splitw-{nc.next_id()}",
                            sync_info=mybir.SyncInfo(on_wait=chunk, on_update=[]),
                            bass_nofuse=True,
                            engine=inst.engine,
                        ))
                    si.on_wait = chunks[-1]
                    inst.sync_info = si
                    n_split += 1
                    changed = True
                out.append(inst)
            if changed:
                try:
                    bb.instructions = out
                except Exception:
                    bb.instructions.clear()
                    for i in out:
                        bb.instructions.append(i)
    return n_split


def _build_program(split=True, reps=1, stages="ABC"):
    from contextlib import ExitStack

    nc = bass.Bass(trn_type="TRN2", target_bir_lowering=False, debug=False)

    dram = dict(
        x=nc.dram_tensor("x", [N, D], F32, kind="ExternalInput").ap(),
        wq=nc.dram_tensor("wq", [D, CW], F32, kind="ExternalInput").ap(),
        wk=nc.dram_tensor("wk", [D, CW], F32, kind="ExternalInput").ap(),
        wv=nc.dram_tensor("wv", [D, CW], F32, kind="ExternalInput").ap(),
        wp=nc.dram_tensor("wp", [CW, D], F32, kind="ExternalInput").ap(),
        qkvb=nc.dram_tensor("qkvb", [3 * CW], F32, kind="ExternalInput").ap(),
        y=nc.dram_tensor("y", [N, D], F32, kind="ExternalOutput").ap(),
    )

    with tile.TileContext(nc) as tc:
        es = ExitStack()
        P = dict(
            const=es.enter_context(tc.tile_pool(name="const", bufs=2)),
            qk=es.enter_context(tc.tile_pool(name="qk", bufs=2)),
            v1=es.enter_context(tc.tile_pool(name="v1", bufs=2)),
            ot=es.enter_context(tc.tile_pool(name="ot", bufs=2)),
            w=es.enter_context(tc.tile_pool(name="w", bufs=1)),
            xT=es.enter_context(tc.tile_pool(name="xT", bufs=2)),
            xg=es.enter_context(tc.tile_pool(name="xg", bufs=2)),
            et=es.enter_context(tc.tile_pool(name="et", bufs=6)),
            ri=es.enter_context(tc.tile_pool(name="ri", bufs=4)),
            otu=es.enter_context(tc.tile_pool(name="otu", bufs=6)),
            y=es.enter_context(tc.tile_pool(name="y", bufs=3)),
            st=es.enter_context(tc.tile_pool(name="ps_st", bufs=2, space="PSUM")),
            us=es.enter_context(tc.tile_pool(name="ps_u", bufs=1, space="PSUM")),
        )

        S = _prologue(nc, P, dram)
        carry = None
        for r in range(reps):
            holder = {}

            def cb(holder=holder, last=(r == reps - 1)):
                if not last:
                    holder["S"] = _prologue(nc, P, dram)

            carry = _compute(nc, P, dram, S, cb, carry, stages)
            S = holder.get("S")
        if carry is not None:
            _final_tail(carry, stages)
        es.close()

    if split:
        _split_sync_waits(nc)
    return nc


def _prologue(nc, P, dram):
    """Allocate one rep's tiles and emit its DMAs/memsets (no PE work)."""
    S = {}
    S["qT"] = P["qk"].tile([128, 2, N], BF, tag="qT", name="qT")
    S["kT"] = P["qk"].tile([128, 2, N], BF, tag="kT", name="kT")
    S["v1"] = P["v1"].tile([128, NT, HL, HD + 1], BF, tag="v1", name="v1")
    S["OT"] = P["ot"].tile([128, 2, N], BF, tag="OT", name="OT")
    S["wq"] = P["w"].tile([128, KC, CW], BF, tag="wq", name="wq_s")
    S["wk"] = P["w"].tile([128, KC, CW], BF, tag="wk", name="wk_s")
    S["wv"] = P["w"].tile([128, KC, CW], BF, tag="wv", name="wv_s")
    S["wp"] = P["w"].tile([128, 2, D], BF, tag="wp", name="wp_s")
    S["qb"] = P["const"].tile([128, 2], F32, tag="qb", name="qb")
    S["kb"] = P["const"].tile([128, 2], F32, tag="kb", name="kb")
    S["vbc"] = P["const"].tile([128, CW], F32, tag="vbc", name="vbc")
    S["ones64"] = P["const"].tile([1, HD], BF, tag="ones", name="ones64")
    warm_in = P["const"].tile([1, 8], F32, tag="wi", name="warm_in")
    warm_out = P["const"].tile([1, 8], F32, tag="wo", name="warm_out")
    S["xT"] = P["xT"].tile([128, KC, N], BF, tag="xT", name="xT")
    xgs = [P["xg"].tile([128, 4, D], BF, tag="xg", name=f"xg{g}")
           for g in range(4)]

    # memsets + act-table warm (off the critical path)
    nc.gpsimd.memset(warm_in, 0.0)
    nc.scalar.activation(warm_out, warm_in,
                         mybir.ActivationFunctionType.Exp)
    nc.gpsimd.memset(S["ones64"], 1.0)
    nc.gpsimd.memset(S["v1"][:, :, :, HD], 1.0)

    # x group casts (f32 -> bf16) + weight casts share the gpsimd SWDGE
    # queue; order so the first-needed tensors land first.
    x_d = dram["x"]

    def emit_cast(g):
        nc.gpsimd.dma_start(
            xgs[g],
            x_d[bass.ds(g * 512, 512), :].rearrange("(t p) c -> p t c", p=128))

    def emit_xbar(g):
        for i in range(4):
            nt = g * 4 + i
            nc.sync.dma_start_transpose(
                S["xT"][:, :, bass.ds(nt * 128, 128)], xgs[g][:, i, :])

    emit_cast(0)
    for (key, ws) in (("wq", S["wq"]), ("wk", S["wk"]), ("wv", S["wv"])):
        nc.gpsimd.dma_start(ws, dram[key].rearrange("(t p) c -> p t c", p=128))
    emit_cast(1)
    emit_cast(2)
    emit_cast(3)
    # wp is read until the END of this rep's tail (which rides the next
    # rep's B00), so its cast can block late; keep it last on the queue.
    nc.gpsimd.dma_start(S["wp"], dram["wp"].rearrange("(t p) e -> p t e", p=128))

    emit_xbar(0)
    qkvb_d = dram["qkvb"]
    for pair in range(2):
        nc.sync.dma_start(S["qb"][:, pair: pair + 1],
                          qkvb_d[bass.ds(pair * 128, 128)].unsqueeze(1))
        nc.sync.dma_start(S["kb"][:, pair: pair + 1],
                          qkvb_d[bass.ds(CW + pair * 128, 128)].unsqueeze(1))
    nc.sync.dma_start(
        S["vbc"],
        qkvb_d[bass.ds(2 * CW, CW)].unsqueeze(0).partition_broadcast(128).squeeze(1))
    emit_xbar(1)
    emit_xbar(2)
    emit_xbar(3)
    return S


def _final_tail(carry, stages="ABC"):
    """Standalone tail for the last rep: finish its last two U stages,
    normalize its B11 (jc-split) and emit the remaining projections."""
    carry["u14"]()
    carry["u15"]()
    carry["recips"]()
    carry["bc_jc"]([(0, 0), (1, 0)])()
    carry["mul_jc"]([(0, 0), (1, 0)])()
    carry["bc_jc"]([(0, 1), (1, 1)])()
    if "C" in stages:
        for nt in range(NT // 2, NT // 2 + 4):
            carry["proj"](nt)
    carry["mul_jc"]([(0, 1), (1, 1)])()
    if "C" in stages:
        for nt in range(NT // 2 + 4, NT):
            carry["proj"](nt)


def _compute(nc, P, dram, S, next_prologue_cb, carry_in, stages="ABC"):
    qT, kT, v1, OT = S["qT"], S["kT"], S["v1"], S["OT"]
    wq_s, wk_s, wv_s, wp_s = S["wq"], S["wk"], S["wv"], S["wp"]
    qb, kb, vbc, ones64, xT = S["qb"], S["kb"], S["vbc"], S["ones64"], S["xT"]
    y_d = dram["y"]

    # ---------------- stage-A emitters ----------------
    def emit_v(mt):
        ps = P["st"].tile([128, 1024], F32, tag="st", name="psv")
        for dc in range(KC):
            nc.tensor.matmul(
                ps[:, 0:CW],
                xT[:, dc, bass.ds(mt * 128, 128)],
                wv_s[:, dc, :],
                start=(dc == 0), stop=(dc == KC - 1))
        nc.vector.tensor_add(
            v1[:, mt, :, 0:HD],
            ps[:, 0:CW].rearrange("p (h d) -> p h d", h=HL),
            vbc.rearrange("p (h d) -> p h d", h=HL))

    def emit_qk(pair, which, nb4):
        wt, dst, bias = ((wq_s, qT, qb), (wk_s, kT, kb))[which]
        ps = P["st"].tile([128, 1024], F32, tag="st", name="psqk")
        for dc in range(KC):
            nc.tensor.matmul(
                ps[:, 0:512],
                wt[:, dc, bass.ds(pair * 128, 128)],
                xT[:, dc, bass.ds(nb4 * 512, 512)],
                start=(dc == 0), stop=(dc == KC - 1))
        nc.vector.tensor_scalar(
            dst[:, pair, bass.ds(nb4 * 512, 512)], ps[:, 0:512],
            bias[:, pair: pair + 1], None, mybir.AluOpType.add)

    # ---------------- stage-B/C emitters ----------------
    def make_us():
        us = {}
        for sub in range(2):
            for jc in range(2):
                us[(sub, jc)] = P["us"].tile([128, 512], F32,
                                             tag=f"u{sub}{jc}",
                                             name=f"u_{sub}_{jc}")
        return us

    def emit_st_exp(pair, nb, mt):
        ets = []
        for sub in range(2):
            st = P["st"].tile([128, 1024], F32, tag="st", name="st")
            for jc in range(2):
                nc.tensor.matmul(
                    st[:, jc * 512:(jc + 1) * 512],
                    kT[bass.ds(sub * HD, HD), pair, bass.ds(mt * 128, 128)],
                    qT[bass.ds(sub * HD, HD), pair,
                       bass.ds(nb * 1024 + jc * 512, 512)],
                    start=True, stop=True)
            et = P["et"].tile([128, 1024], BF, tag="et", name="et")
            nc.scalar.activation(
                et, st, mybir.ActivationFunctionType.Exp, scale=float(SCALE))
            ets.append(et)
        return ets

    def emit_u(pair, us, mt, ets):
        for sub in range(2):
            for jc in range(2):
                nc.tensor.matmul(
                    us[(sub, jc)][0:HD + 1, :],
                    v1[:, mt, pair * 2 + sub, :],
                    ets[sub][:, jc * 512:(jc + 1) * 512],
                    start=(mt == 0), stop=(mt == NT - 1))

    def emit_recips(us, ris, otus):
        """Reciprocal of the denominator rows + copy of the numerator
        rows to SBUF (the DVE may read only ONE operand from PSUM, so
        the normalize multiply needs the numerators in SBUF)."""
        for sub in range(2):
            for jc in range(2):
                ri = P["ri"].tile([1, 512], BF, tag="ri", name="ri")
                with nc.allow_low_precision(reason="bf16 reciprocal feeds the bf16 broadcast matmul"):
                    nc.vector.reciprocal(ri, us[(sub, jc)][HD:HD + 1, :])
                ris[(sub, jc)] = ri
                otu = P["otu"].tile([HD, 512], F32, tag="otu", name="otu")
                nc.vector.tensor_copy(otu, us[(sub, jc)][0:HD, :])
                otus[(sub, jc)] = otu

    def emit_bc(us, ris, keys):
        for key in keys:
            nc.tensor.matmul(
                us[key][HD:128, :], ones64, ris[key],
                start=True, stop=True)

    def emit_norm_muls(pair, nb, us, otus, keys):
        for (sub, jc) in keys:
            nc.vector.tensor_mul(
                OT[bass.ds(sub * HD, HD), pair,
                   bass.ds(nb * 1024 + jc * 512, 512)],
                otus[(sub, jc)], us[(sub, jc)][HD:128, :])

    def emit_proj(nt):
        ps = P["st"].tile([128, 1024], F32, tag="st", name="psy")
        yt = P["y"].tile([128, D], F32, tag="y", name="y")
        for ec in range(2):
            for pair in range(2):
                nc.tensor.matmul(
                    ps[:, bass.ds(ec * 512, 512)],
                    OT[:, pair, bass.ds(nt * 128, 128)],
                    wp_s[:, pair, bass.ds(ec * 512, 512)],
                    start=(pair == 0), stop=(pair == 1))
            nc.vector.tensor_copy(yt[:, bass.ds(ec * 512, 512)],
                                  ps[:, bass.ds(ec * 512, 512)])
        nc.sync.dma_start(y_d[bass.ds(nt * 128, 128), :], yt)

    # ---------------- emission schedule (flat 64-slot stream) --------
    # The whole rep is ONE pipeline of 64 ST stages (4 blocks x 16 mts,
    # slot s = 16*b + mt). U(s) is emitted after ST(s+2) so the PE never
    # sits behind the ACT exp latency; block/rep boundaries are just
    # scheduled atoms (prev-block normalization at slots +1/+2), and
    # stage-A leftovers + projection tiles fill the ACT-paced slack.
    BL = [(0, 0), (0, 1), (1, 0), (1, 1)]

    if "B" not in stages:
        emit_qk(0, 0, 0)
        emit_qk(0, 1, 0)
        emit_v(0)
        emit_qk(0, 0, 1)
        for mt in range(1, NT):
            emit_v(mt)
        for nb4 in range(2, 4):
            emit_qk(0, 0, nb4)
        for nb4 in range(1, 4):
            emit_qk(0, 1, nb4)
        for which in range(2):
            for nb4 in range(4):
                emit_qk(1, which, nb4)
        next_prologue_cb()
        return None

    uss = [make_us() for _ in range(4)]
    riss = [None] * 4
    otuss = [None] * 4

    pre = {s: [] for s in range(64)}
    post = {s: [] for s in range(64)}
    after = {s: [] for s in range(64)}

    # prefix: operands of ST(0) (and the first U)
    pre[0] += [lambda: emit_qk(0, 0, 0), lambda: emit_qk(0, 1, 0),
               lambda: emit_qk(0, 0, 1)]
    after[0].append(lambda: emit_v(0))
    for j in range(1, NT):
        after[j].append(lambda m=j: emit_v(m))
    for k in range(1, 4):
        pre[4 * k].append(lambda n=k: emit_qk(0, 1, n))
    for k in range(4):
        pre[32 + 4 * k].append(lambda n=k: emit_qk(1, 1, n))

    def sched_norm(b):
        """Normalization of block b rides slots 16(b+1)+1 / +2."""
        base = 16 * (b + 1)
        after[base + 1].append(
            lambda: riss.__setitem__(b, None) or None)

    # recips/bc/muls for in-rep block boundaries
    def mk_recips(b):
        def fn():
            ris, otus = {}, {}
            emit_recips(uss[b], ris, otus)
            riss[b] = ris
            otuss[b] = otus
        return fn

    def mk_bc_muls(b):
        def fn():
            pair, nb = BL[b]
            emit_bc(uss[b], riss[b], ALL4)
            emit_norm_muls(pair, nb, uss[b], otuss[b], ALL4)
        return fn

    for b in range(3):
        after[16 * (b + 1) + 1].append(mk_recips(b))
        post[16 * (b + 1) + 2].append(mk_bc_muls(b))

    # carry-in: previous rep's B11 finishes inside this rep's first slots
    if carry_in is not None:
        post[0].append(carry_in["u14"])
        post[1].append(carry_in["u15"])
        after[1].append(carry_in["recips"])
        post[2].append(carry_in["bc_muls"])

    # fillers: (avail_slot, deadline_slot_or_None, fn)
    fillers = []
    fillers.append([0, 16, lambda: emit_qk(0, 0, 2)])
    fillers.append([0, 16, lambda: emit_qk(0, 0, 3)])
    fillers.append([0, 32, lambda: emit_qk(1, 0, 0)])
    fillers.append([0, 32, lambda: emit_qk(1, 0, 1)])
    fillers.append([0, 48, lambda: emit_qk(1, 0, 2)])
    fillers.append([0, 48, lambda: emit_qk(1, 0, 3)])
    if carry_in is not None and "C" in stages:
        for nt in range(NT // 2, NT):
            fillers.append([3, None, lambda n=nt, p=carry_in["proj"]: p(n)])
    if "C" in stages:
        for nt in range(NT // 2):
            fillers.append([51, None, lambda n=nt: emit_proj(n)])

    def pull_filler(s, limit=1):
        n = 0
        for ent in list(fillers):
            if n >= limit:
                break
            if ent[0] <= s:
                fillers.remove(ent)
                ent[2]()
                n += 1
        return n

    pend = {}
    for s in range(64):
        b, mt = divmod(s, 16)
        pair, nb = BL[b]
        # forced: deadline fillers + pre atoms
        for ent in list(fillers):
            if ent[1] is not None and ent[1] <= s:
                fillers.remove(ent)
                ent[2]()
        for fn in pre[s]:
            fn()
        pend[s] = emit_st_exp(pair, nb, mt)
        for fn in post[s]:
            fn()
        if s % 16 == 2:
            pull_filler(s)  # cover the U(b,0) wait on the boundary muls
        if s - 2 >= 0:
            pb, pmt = divmod(s - 2, 16)
            emit_u(BL[pb][0], uss[pb], pmt, pend.pop(s - 2))
        for fn in after[s]:
            fn()
        pull_filler(s)
    while fillers:
        fillers.pop(0)[2]()

    # carry for the next rep (or the final tail)
    def mk_u(s):
        pb, pmt = divmod(s, 16)
        ets = pend.pop(s)
        return lambda: emit_u(BL[pb][0], uss[pb], pmt, ets)

    def mk_final_recips():
        def fn():
            ris, otus = {}, {}
            emit_recips(uss[3], ris, otus)
            riss[3] = ris
            otuss[3] = otus
        return fn

    def mk_final_bc_muls():
        def fn():
            emit_bc(uss[3], riss[3], ALL4)
            emit_norm_muls(1, 1, uss[3], otuss[3], ALL4)
        return fn

    def mk_bc_keys(keys):
        def fn():
            emit_bc(uss[3], riss[3], keys)
        return fn

    def mk_mul_keys(keys):
        def fn():
            emit_norm_muls(1, 1, uss[3], otuss[3], keys)
        return fn

    return {
        "u14": mk_u(62),
        "u15": mk_u(63),
        "recips": mk_final_recips(),
        "bc_muls": mk_final_bc_muls(),
        "bc_jc": mk_bc_keys,
        "mul_jc": mk_mul_keys,
        "proj": emit_proj,
    }


_NC_CACHE = None


def _get_program():
    global _NC_CACHE
    if _NC_CACHE is None:
        _NC_CACHE = _build_program()
    return _NC_CACHE


def make_in_maps(x, qkv_w, qkv_b, proj_w):
    in_maps = []
    for c in range(NC):
        b, j = divmod(c, NC // B)
        cs = j * CW
        in_maps.append({
            "x": np.ascontiguousarray(x[b], np.float32),
            "wq": np.ascontiguousarray(qkv_w[:, cs: cs + CW], np.float32),
            "wk": np.ascontiguousarray(qkv_w[:, D + cs: D + cs + CW], np.float32),
            "wv": np.ascontiguousarray(qkv_w[:, 2 * D + cs: 2 * D + cs + CW], np.float32),
            "wp": np.ascontiguousarray(proj_w[cs: cs + CW, :], np.float32),
            "qkvb": np.concatenate([
                qkv_b[cs: cs + CW],
                qkv_b[D + cs: D + cs + CW],
                qkv_b[2 * D + cs: 2 * D + cs + CW]]).astype(np.float32),
        })
    return in_maps


def combine_outputs(results, proj_b):
    out = np.empty((B, N, D), np.float32)
    per = NC // B
    for b in range(B):
        acc = results[b * per]["y"].astype(np.float32)
        for c in range(b * per + 1, (b + 1) * per):
            acc = acc + results[c]["y"]
        out[b] = acc + proj_b[None, :].astype(np.float32)
    return out


def kernel(**inputs):
    x = np.asarray(inputs["x"], np.float32)
    qkv_w = np.asarray(inputs["qkv_w"], np.float32)
    qkv_b = np.asarray(inputs["qkv_b"], np.float32)
    proj_w = np.asarray(inputs["proj_w"], np.float32)
    proj_b = np.asarray(inputs["proj_b"], np.float32)

    nc = _get_program()
    in_maps = make_in_maps(x, qkv_w, qkv_b, proj_w)
    res = run_bass_kernel_spmd(nc, in_maps, list(range(NC)), trace=False)
    return combine_outputs(res.results, proj_b)
